# revision 26
# baseline (speedup 1.0000x reference)
"""Causal self-attention (B=2, T=2048, C=2048, H=16) on 8 TRN2 NeuronCores.

Sharding: data-parallel over batch (2) x tensor-parallel over heads (4 heads
per core). Each core computes, for its batch element b and head group g:
  QKV projection for its heads' columns, causal attention for its 4 heads,
  and a partial output projection (row-sharded W_proj). The host sums the
  4 partial projections per batch element.

Key performance structure:
  - All inputs host-prepacked into partition-major layouts so every DMA
    moves >=2KB contiguous per partition line, issued in first-use order
    on the sync HWDGE ring. The first K-head weight group and x chunk are
    split into half-size DMAs so the first projection chain starts after
    ~1.25MB; its first ~13 cold matmuls double as the PE HAM clock-gate
    warm-up (2.4 GHz reached ~6us in).
  - Software-pipelined chunk schedule: chunk j's QKV projection groups are
    interleaved at ~4us granularity with attention SEGMENTS (4 key-tiles)
    of chunk j-1 and the output projection of chunk j-2. Each segment is
    further split into an S phase (S^T matmuls + exp + mask) and an AV
    phase emitted one GEMM group later, so the ACT exp latency is hidden
    under a full ~4.3us projection group. Every cross-engine dependency
    (PSUM->DVE qkt copy, ACT exp backlog, DMA transpose) is at least one
    GEMM group old by the time the PE consumes it, so the PE never waits.
    Chunk 3 emits its Q groups FIRST so all of chunk 3's attention (whose
    kt<=11 tiles only need K/V from chunks 0..2) interleaves there too,
    with quarter-tiles of the chunk-1 output projection as PE cover for
    the ACT exp backlog; the tail is pure projection streaming.
  - The softmax accumulators for 4 q-subtiles live as 2 PSUM bank pairs;
    only the first accumulator of a pair issues start=True (the bank clear
    also zeroes its bank-mate, which accumulates from zero).
  - The y^T transpose needed by the output projection runs on the DMA xbar
    (dma_start_transpose), not the PE.

Per-core device layouts (fp16 compute / fp32 PSUM accumulation):
  xb   [128, 4, 16, 512]  x^T tiles, chunk-major: [p, tj, c, t]
  wqk  [128, 8, 16, 128]  [p, coltile, c, n]; coltiles 0..3 Q heads, 4..7 K
  wv   [128, 16, 512]     [p, c, (h d)]
  wp   [128, 4, 2048]     [p, h, c]  W_proj rows for this head group
  out  [T, C] fp16 partial projection output

Attention per (head, 512-wide q-chunk): S^T = K_kt^T.T @ Q^T per key tile,
P^T = exp(scale*S^T) (ACT), diagonal masks on DVE, Y[q, d+1] += P^T.T @
[V | ones] accumulated in PSUM (the ones column gives the softmax
denominator), y = Y[:, :d] * (1/Y[:, d]) on DVE, then DMA-transpose into
yt[d, h, t] for the projection.
"""

import os

import numpy as np

N_HEAD = 16
N_EMBD = 2048
B = 2
T = 2048
C = N_EMBD
D = C // N_HEAD  # 128
HPC = N_HEAD // 4  # heads per core = 4
N_CORES = 8
CT = C // 128  # 16 contraction tiles
TT = T // 128  # 16 t tiles
NCH = T // 512  # 4 chunks of 512

LAST_EXEC_NS = None

_CACHE = {}


def _build_nc():
    import concourse.bass as bass  # noqa: F401
    import concourse.tile as tile
    from concourse import bacc, mybir

    F32 = mybir.dt.float32
    F16 = mybir.dt.float16
    Exp = mybir.ActivationFunctionType.Exp
    Copy = mybir.ActivationFunctionType.Copy
    SCALE = 1.0 / float(np.sqrt(D))

    nc = bacc.Bacc("TRN2", target_bir_lowering=False, num_devices=N_CORES)

    xb_d = nc.dram_tensor("xb", [128, NCH, CT, 512], F16, kind="ExternalInput")
    wqk_d = nc.dram_tensor("wqk", [128, 8, CT, 128], F16, kind="ExternalInput")
    wv_d = nc.dram_tensor("wv", [128, CT, 512], F16, kind="ExternalInput")
    wp_d = nc.dram_tensor("wp", [128, HPC, C], F16, kind="ExternalInput")
    out_d = nc.dram_tensor("out_part", [T, C], F16, kind="ExternalOutput")

    # Diagonal causal masks, partition-major: [128 k, diag idx, 512 q].
    kk = np.arange(128)[:, None]
    qq = np.arange(512)[None, :]
    masks = np.stack(
        [(qq >= (128 * i + kk)).astype(np.float16) for i in range(4)], axis=1
    )  # [128, 4, 512]
    masks_d = nc.inline_tensor(np.ascontiguousarray(masks), name="diagmasks")
    ident_d = nc.inline_tensor(np.eye(128, dtype=np.float16), name="ident128")

    with tile.TileContext(nc) as tc:
        with (
            tc.tile_pool(name="singles", bufs=1) as singles,
            tc.tile_pool(name="xbp", bufs=3) as xbp,
            tc.tile_pool(name="ptp", bufs=10) as ptp,
            tc.tile_pool(name="ysb", bufs=4) as ysbp,
            tc.tile_pool(name="rp", bufs=4) as rp,
            tc.tile_pool(name="ost", bufs=3) as ostp,
            tc.tile_pool(name="ps", bufs=6, space="PSUM") as ps,
            tc.tile_pool(name="yps", bufs=2, space="PSUM") as yps,
        ):
            # ---- Input DMAs, issued in first-use order on the sync ring ----
            wqk_sb = singles.tile([128, 8, CT, 128], F16, name="wqk_sb")
            wv_sb = singles.tile([128, CT, 512], F16, name="wv_sb")
            wp_sb = singles.tile([128, HPC, C], F16, name="wp_sb")
            mask_sb = singles.tile([128, 4, 512], F16, name="mask_sb")
            xb_t = [None] * NCH

            xb_t[0] = xbp.tile([128, CT, 512], F16, tag="xb", name="xb0")
            # first K group + x chunk 0, split in halves: the first 8 matmuls
            # of the first chain start after ~1.25MB instead of 2.5MB (these
            # cold matmuls also warm the PE HAM clock-gate)
            nc.sync.dma_start(out=wqk_sb[:, 4, 0:2], in_=wqk_d[:, 4, 0:2])
            nc.sync.dma_start(out=xb_t[0][:, 0:2], in_=xb_d[:, 0, 0:2])
            nc.sync.dma_start(out=wqk_sb[:, 4, 2:8], in_=wqk_d[:, 4, 2:8])
            nc.sync.dma_start(out=xb_t[0][:, 2:8], in_=xb_d[:, 0, 2:8])
            nc.sync.dma_start(out=wqk_sb[:, 4, 8:16], in_=wqk_d[:, 4, 8:16])
            nc.sync.dma_start(out=xb_t[0][:, 8:16], in_=xb_d[:, 0, 8:16])
            for ct in (5, 6, 7):
                nc.sync.dma_start(out=wqk_sb[:, ct], in_=wqk_d[:, ct])
            nc.sync.dma_start(out=wv_sb, in_=wv_d[:, :])
            for ct in (0, 1, 2, 3):
                nc.sync.dma_start(out=wqk_sb[:, ct], in_=wqk_d[:, ct])
            nc.sync.dma_start(out=mask_sb, in_=masks_d[:, :, :])
            ident_sb = singles.tile([128, 128], F16, name="ident_sb")
            nc.sync.dma_start(out=ident_sb, in_=ident_d[:, :])
            nc.sync.dma_start(out=wp_sb, in_=wp_d[:, :])
            xb_t[1] = xbp.tile([128, CT, 512], F16, tag="xb", name="xb1")
            nc.sync.dma_start(out=xb_t[1], in_=xb_d[:, 1])

            # qkt: [d, coltile, t]; coltiles 0..3 = Q heads, 4..7 = K heads
            qkt_sb = singles.tile([128, 8, T], F16)
            # v with a ones column per (kt, head): [kt-tile, head, 129]
            vv_sb = singles.tile([128, TT, HPC, 129], F16)
            # y transposed: [d, head, t]
            yt_sb = singles.tile([128, HPC, T], F16)

            def qkv_group(tj, ct):
                # projection group for coltile ct of chunk tj (N=512, 16 MMs)
                xt = xb_t[tj]
                pq = ps.tile([128, 512], F32, tag="ps", name=f"pq{tj}_{ct}")
                for c in range(CT):
                    nc.tensor.matmul(
                        pq,
                        wqk_sb[:, ct, c, :],
                        xt[:, c, :],
                        start=(c == 0),
                        stop=(c == CT - 1),
                    )
                nc.vector.tensor_copy(
                    out=qkt_sb[:, ct, tj * 512 : (tj + 1) * 512], in_=pq
                )

            def v_group(tj, tt):
                kt = tj * 4 + tt
                xt = xb_t[tj]
                pv = ps.tile([128, 512], F32, tag="ps", name=f"pv{kt}")
                for c in range(CT):
                    nc.tensor.matmul(
                        pv,
                        xt[:, c, tt * 128 : (tt + 1) * 128],
                        wv_sb[:, c, :],
                        start=(c == 0),
                        stop=(c == CT - 1),
                    )
                nc.vector.tensor_copy(
                    out=vv_sb[:, kt, :, 0:128],
                    in_=pv.rearrange("p (h d) -> p h d", h=HPC),
                )
                nc.vector.memset(vv_sb[:, kt, :, 128:129], 1.0)

            y_live = {}  # h -> y_tiles for the attention chunk in flight

            pt_live = {}  # (h, kt) -> exp'd P^T tile awaiting its AV phase

            def attn_s(j, h, sg):
                # S + exp + mask for segment sg (4 kt); AV runs later so the
                # ACT exp backlog is covered by interleaved GEMM groups
                if sg == 0:
                    pairs = [
                        yps.tile([128, 258], F32, tag="y", name=f"yp{h}_{j}_{q}")
                        for q in range(2)
                    ]
                    y_live[h] = [(pairs[qs // 2], (qs % 2) * 129) for qs in range(4)]
                for kt in range(4 * sg, 4 * sg + 4):
                    di = kt - 4 * j
                    lo = 128 * di if di > 0 else 0
                    ss = ps.tile([128, 512], F32, tag="ps", name=f"ss{h}{j}{kt}")
                    nc.tensor.matmul(
                        ss[:, lo:],
                        qkt_sb[:, 4 + h, kt * 128 : (kt + 1) * 128],
                        qkt_sb[:, h, j * 512 + lo : (j + 1) * 512],
                        start=True,
                        stop=True,
                    )
                    pt = ptp.tile([128, 512], F16, tag="pt", name=f"pt{h}{j}{kt}")
                    nc.scalar.activation(
                        out=pt[:, lo:], in_=ss[:, lo:], func=Exp, scale=SCALE
                    )
                    if di >= 0:
                        nc.vector.tensor_mul(
                            pt[:, lo : lo + 128],
                            pt[:, lo : lo + 128],
                            mask_sb[:, di, lo : lo + 128],
                        )
                    pt_live[(h, kt)] = pt

            def attn_av(j, h, sg, final=False):
                y_tiles = y_live[h]
                for kt in range(4 * sg, 4 * sg + 4):
                    di = kt - 4 * j
                    pt = pt_live.pop((h, kt))
                    for qs in range(max(0, di), 4):
                        # paired accumulators share a PSUM bank; start=True
                        # clears the whole bank, so only the off==0 group may
                        # use it (the clear also zeroes its bank-mate, which
                        # then accumulates from zero with start=False).
                        yp, off = y_tiles[qs]
                        nc.tensor.matmul(
                            yp[:, off : off + 129],
                            pt[:, qs * 128 : (qs + 1) * 128],
                            vv_sb[:, kt, h, :],
                            start=(kt == 0 and off == 0),
                            stop=(kt == 4 * j + qs),
                            skip_group_check=(off != 0),
                        )
                if final:
                    attn_head_end(j, h)

            def attn_head_end(j, h, on_pe=False):
                y_tiles = y_live.pop(h)
                for qs in range(4):
                    yp, off = y_tiles[qs]
                    r = rp.tile([128, 1], F32, tag="r", name=f"r{h}{j}{qs}")
                    nc.vector.reciprocal(r, yp[:, off + 128 : off + 129])
                    y16 = ysbp.tile([128, 128], F16, tag="y16", name=f"y16_{qs}")
                    nc.vector.tensor_scalar_mul(y16, yp[:, off : off + 128], r)
                    tglob = (j * 4 + qs) * 128
                    if on_pe:
                        # final head: PE transpose avoids the DMA-xbar latency
                        # right before the projection consumes yt
                        ytp = ps.tile([128, 128], F16, tag="ps", name=f"ytp{qs}")
                        nc.tensor.transpose(ytp, y16, ident_sb)
                        nc.vector.tensor_copy(
                            out=yt_sb[:, h, tglob : tglob + 128], in_=ytp
                        )
                    else:
                        nc.sync.dma_start_transpose(
                            out=yt_sb[:, h, tglob : tglob + 128], in_=y16
                        )

            ot_tiles = {}

            def proj_piece(tt, cc):
                # one quarter (512 out cols) of an output-projection tile,
                # copies on DVE; used to interleave with attention segments
                if cc == 0:
                    ot_tiles[tt] = ostp.tile([128, C], F16, tag="ot", name=f"ot{tt}")
                ot = ot_tiles[tt]
                po = ps.tile([128, 512], F32, tag="ps", name=f"po{tt}_{cc}")
                for hd in range(HPC):
                    nc.tensor.matmul(
                        po,
                        yt_sb[:, hd, tt * 128 : (tt + 1) * 128],
                        wp_sb[:, hd, cc * 512 : (cc + 1) * 512],
                        start=(hd == 0),
                        stop=(hd == HPC - 1),
                    )
                nc.vector.tensor_copy(out=ot[:, cc * 512 : (cc + 1) * 512], in_=po)
                if cc % 2 == 1:
                    nc.sync.dma_start(
                        out=out_d[
                            tt * 128 : (tt + 1) * 128,
                            (cc - 1) * 512 : (cc + 1) * 512,
                        ],
                        in_=ot[:, (cc - 1) * 512 : (cc + 1) * 512],
                    )
                if cc == 3:
                    ot_tiles.pop(tt)

            def proj_tile(tt, fine_store=False, dve_copies=False):
                ot = ostp.tile([128, C], F16, tag="ot", name=f"ot{tt}")
                for cc in range(4):
                    po = ps.tile([128, 512], F32, tag="ps", name=f"po{tt}_{cc}")
                    for hd in range(HPC):
                        nc.tensor.matmul(
                            po,
                            yt_sb[:, hd, tt * 128 : (tt + 1) * 128],
                            wp_sb[:, hd, cc * 512 : (cc + 1) * 512],
                            start=(hd == 0),
                            stop=(hd == HPC - 1),
                        )
                    if dve_copies or cc % 2 == 0:
                        nc.vector.tensor_copy(
                            out=ot[:, cc * 512 : (cc + 1) * 512], in_=po
                        )
                    else:
                        nc.scalar.activation(
                            out=ot[:, cc * 512 : (cc + 1) * 512], in_=po, func=Copy
                        )
                    if fine_store:
                        nc.sync.dma_start(
                            out=out_d[
                                tt * 128 : (tt + 1) * 128,
                                cc * 512 : (cc + 1) * 512,
                            ],
                            in_=ot[:, cc * 512 : (cc + 1) * 512],
                        )
                    elif cc % 2 == 1:
                        nc.sync.dma_start(
                            out=out_d[
                                tt * 128 : (tt + 1) * 128,
                                (cc - 1) * 512 : (cc + 1) * 512,
                            ],
                            in_=ot[:, (cc - 1) * 512 : (cc + 1) * 512],
                        )

            def chunk_groups(tj):
                for ct in (4, 5, 6, 7):
                    yield ("qkv", ct)
                for tt in range(4):
                    yield ("v", tt)
                for h in range(HPC):
                    yield ("qkv", h)

            # ---- chunk 0: pure QKV ----
            for kind, a in chunk_groups(0):
                (qkv_group if kind == "qkv" else v_group)(0, a)

            # ---- chunks 1..2: QKV(j) x attn(j-1) segments x proj(j-2) ----
            for j in range(1, NCH - 1):
                if j + 1 < NCH:
                    xb_t[j + 1] = xbp.tile(
                        [128, CT, 512], F16, tag="xb", name=f"xb{j + 1}"
                    )
                    nc.sync.dma_start(out=xb_t[j + 1], in_=xb_d[:, j + 1])
                aj = j - 1
                segs = [(h, s) for h in range(HPC) for s in range(aj + 1)]
                s_i = 0
                av_i = 0
                for gi, (kind, a) in enumerate(chunk_groups(j)):
                    (qkv_group if kind == "qkv" else v_group)(j, a)
                    while av_i < gi * len(segs) // 12:
                        h, s = segs[av_i]
                        attn_av(aj, h, s, final=(s == aj))
                        av_i += 1
                    while s_i < (gi + 1) * len(segs) // 12:
                        attn_s(aj, *segs[s_i])
                        s_i += 1
                while av_i < len(segs):
                    h, s = segs[av_i]
                    attn_av(aj, h, s, final=(s == aj))
                    av_i += 1
                if j >= 2:
                    for tt in range(4 * (j - 2), 4 * (j - 2) + 4):
                        proj_tile(tt)

            # ---- chunk 3: Q groups first, then K/V, x attn(2) segments.
            # attn(3) segs 0..2 only need K/V from chunks 0..2, so after the
            # Q groups the whole of attn(3) interleaves here, leaving only
            # pure projection work for the tail.
            j = NCH - 1
            aj = j - 1
            segs = [(h, s) for h in range(HPC) for s in range(aj + 1)]
            s_i = 0
            av_i = 0
            c3_groups = [("qkv", h) for h in range(HPC)]
            c3_groups += [("qkv", ct) for ct in (4, 5, 6, 7)]
            c3_groups += [("v", tt) for tt in range(4)]
            for gi, (kind, a) in enumerate(c3_groups):
                (qkv_group if kind == "qkv" else v_group)(j, a)
                while av_i < gi * len(segs) // 12:
                    h, s = segs[av_i]
                    attn_av(aj, h, s, final=(s == aj))
                    av_i += 1
                while s_i < (gi + 1) * len(segs) // 12:
                    attn_s(aj, *segs[s_i])
                    s_i += 1
            while av_i < len(segs):
                h, s = segs[av_i]
                attn_av(aj, h, s, final=(s == aj))
                av_i += 1
            # attn(3) heads with a proj(1) quarter-tile after every segment
            # as PE cover for the ACT exp backlog
            pieces = [
                (4 * (NCH - 3) + t2, cc) for t2 in range(4) for cc in range(4)
            ]
            pi = 0
            prev = None
            for h in range(HPC):
                for s in range(NCH):
                    attn_s(NCH - 1, h, s)
                    if pi < len(pieces):
                        proj_piece(*pieces[pi])
                        pi += 1
                    if prev is not None:
                        ph, psg = prev
                        attn_av(NCH - 1, ph, psg, final=(psg == NCH - 1))
                    prev = (h, s)
            attn_av(NCH - 1, prev[0], prev[1], final=True)
            # pure projection tail: proj(2) then proj(3)
            for tt in range(4 * (NCH - 2), 4 * NCH):
                proj_tile(tt, fine_store=(tt == 4 * NCH - 1))

    nc.compile()
    return nc


# revision 27
# speedup vs baseline: 1.0098x; 1.0098x over previous
"""Causal self-attention (B=2, T=2048, C=2048, H=16) on 8 TRN2 NeuronCores.

Sharding: data-parallel over batch (2) x tensor-parallel over heads (4 heads
per core). Each core computes, for its batch element b and head group g:
  QKV projection for its heads' columns, causal attention for its 4 heads,
  and a partial output projection (row-sharded W_proj). The host sums the
  4 partial projections per batch element.

Key performance structure:
  - All inputs host-prepacked into partition-major layouts so every DMA
    moves >=2KB contiguous per partition line, issued in first-use order
    on the sync HWDGE ring. The first K-head weight group and x chunk are
    split into half-size DMAs so the first projection chain starts after
    ~1.25MB; its first ~13 cold matmuls double as the PE HAM clock-gate
    warm-up (2.4 GHz reached ~6us in).
  - Software-pipelined chunk schedule: chunk j's QKV projection groups are
    interleaved at ~4us granularity with attention SEGMENTS (4 key-tiles)
    of chunk j-1 and the output projection of chunk j-2. Each segment is
    further split into an S phase (S^T matmuls + exp + mask) and an AV
    phase emitted one GEMM group later, so the ACT exp latency is hidden
    under a full ~4.3us projection group. Every cross-engine dependency
    (PSUM->DVE qkt copy, ACT exp backlog, DMA transpose) is at least one
    GEMM group old by the time the PE consumes it, so the PE never waits.
    Chunk 3 emits its Q groups FIRST so all of chunk 3's attention (whose
    kt<=11 tiles only need K/V from chunks 0..2) interleaves there too,
    with quarter-tiles of the chunk-1 output projection as PE cover for
    the ACT exp backlog; the tail is pure projection streaming.
  - The softmax accumulators for 4 q-subtiles live as 2 PSUM bank pairs;
    only the first accumulator of a pair issues start=True (the bank clear
    also zeroes its bank-mate, which accumulates from zero).
  - The y^T transpose needed by the output projection runs on the DMA xbar
    (dma_start_transpose), not the PE.

Per-core device layouts (fp16 compute / fp32 PSUM accumulation):
  xb   [128, 4, 16, 512]  x^T tiles, chunk-major: [p, tj, c, t]
  wqk  [128, 8, 16, 128]  [p, coltile, c, n]; coltiles 0..3 Q heads, 4..7 K
  wv   [128, 16, 512]     [p, c, (h d)]
  wp   [128, 4, 2048]     [p, h, c]  W_proj rows for this head group
  out  [T, C] fp16 partial projection output

Attention per (head, 512-wide q-chunk): S^T = K_kt^T.T @ Q^T per key tile,
P^T = exp(scale*S^T) (ACT), diagonal masks on DVE, Y[q, d+1] += P^T.T @
[V | ones] accumulated in PSUM (the ones column gives the softmax
denominator), y = Y[:, :d] * (1/Y[:, d]) on DVE, then DMA-transpose into
yt[d, h, t] for the projection.
"""

import os

import numpy as np

N_HEAD = 16
N_EMBD = 2048
B = 2
T = 2048
C = N_EMBD
D = C // N_HEAD  # 128
HPC = N_HEAD // 4  # heads per core = 4
N_CORES = 8
CT = C // 128  # 16 contraction tiles
TT = T // 128  # 16 t tiles
NCH = T // 512  # 4 chunks of 512

LAST_EXEC_NS = None

_CACHE = {}


def _build_nc():
    import concourse.bass as bass  # noqa: F401
    import concourse.tile as tile
    from concourse import bacc, mybir

    F32 = mybir.dt.float32
    F16 = mybir.dt.float16
    Exp = mybir.ActivationFunctionType.Exp
    Copy = mybir.ActivationFunctionType.Copy
    SCALE = 1.0 / float(np.sqrt(D))

    nc = bacc.Bacc("TRN2", target_bir_lowering=False, num_devices=N_CORES)

    xb_d = nc.dram_tensor("xb", [128, NCH, CT, 512], F16, kind="ExternalInput")
    wqk_d = nc.dram_tensor("wqk", [128, 8, CT, 128], F16, kind="ExternalInput")
    wv_d = nc.dram_tensor("wv", [128, CT, 512], F16, kind="ExternalInput")
    wp_d = nc.dram_tensor("wp", [128, HPC, C], F16, kind="ExternalInput")
    out_d = nc.dram_tensor("out_part", [T, C], F16, kind="ExternalOutput")

    # Diagonal causal masks, partition-major: [128 k, diag idx, 512 q].
    kk = np.arange(128)[:, None]
    qq = np.arange(512)[None, :]
    masks = np.stack(
        [(qq >= (128 * i + kk)).astype(np.float16) for i in range(4)], axis=1
    )  # [128, 4, 512]
    masks_d = nc.inline_tensor(np.ascontiguousarray(masks), name="diagmasks")
    ident_d = nc.inline_tensor(np.eye(128, dtype=np.float16), name="ident128")

    with tile.TileContext(nc) as tc:
        with (
            tc.tile_pool(name="singles", bufs=1) as singles,
            tc.tile_pool(name="xbp", bufs=3) as xbp,
            tc.tile_pool(name="ptp", bufs=10) as ptp,
            tc.tile_pool(name="ysb", bufs=4) as ysbp,
            tc.tile_pool(name="rp", bufs=4) as rp,
            tc.tile_pool(name="ost", bufs=3) as ostp,
            tc.tile_pool(name="ps", bufs=6, space="PSUM") as ps,
            tc.tile_pool(name="yps", bufs=2, space="PSUM") as yps,
        ):
            # ---- Input DMAs, issued in first-use order on the sync ring ----
            wqk_sb = singles.tile([128, 8, CT, 128], F16, name="wqk_sb")
            wv_sb = singles.tile([128, CT, 512], F16, name="wv_sb")
            wp_sb = singles.tile([128, HPC, C], F16, name="wp_sb")
            mask_sb = singles.tile([128, 4, 512], F16, name="mask_sb")
            xb_t = [None] * NCH

            xb_t[0] = xbp.tile([128, CT, 512], F16, tag="xb", name="xb0")
            # first K group + x chunk 0, split in halves: the first 8 matmuls
            # of the first chain start after ~1.25MB instead of 2.5MB (these
            # cold matmuls also warm the PE HAM clock-gate)
            nc.sync.dma_start(out=wqk_sb[:, 4, 0:8], in_=wqk_d[:, 4, 0:8])
            nc.sync.dma_start(out=xb_t[0][:, 0:8], in_=xb_d[:, 0, 0:8])
            nc.sync.dma_start(out=wqk_sb[:, 4, 8:16], in_=wqk_d[:, 4, 8:16])
            nc.sync.dma_start(out=xb_t[0][:, 8:16], in_=xb_d[:, 0, 8:16])
            for ct in (5, 6, 7):
                nc.sync.dma_start(out=wqk_sb[:, ct], in_=wqk_d[:, ct])
            nc.sync.dma_start(out=wv_sb, in_=wv_d[:, :])
            for ct in (0, 1, 2, 3):
                nc.sync.dma_start(out=wqk_sb[:, ct], in_=wqk_d[:, ct])
            nc.sync.dma_start(out=mask_sb, in_=masks_d[:, :, :])
            ident_sb = singles.tile([128, 128], F16, name="ident_sb")
            nc.sync.dma_start(out=ident_sb, in_=ident_d[:, :])
            nc.sync.dma_start(out=wp_sb, in_=wp_d[:, :])
            xb_t[1] = xbp.tile([128, CT, 512], F16, tag="xb", name="xb1")
            nc.sync.dma_start(out=xb_t[1], in_=xb_d[:, 1])

            # qkt: [d, coltile, t]; coltiles 0..3 = Q heads, 4..7 = K heads
            qkt_sb = singles.tile([128, 8, T], F16)
            # v with a ones column per (kt, head): [kt-tile, head, 129]
            vv_sb = singles.tile([128, TT, HPC, 129], F16)
            # y transposed: [d, head, t]
            yt_sb = singles.tile([128, HPC, T], F16)

            def qkv_group(tj, ct):
                # projection group for coltile ct of chunk tj (N=512, 16 MMs)
                xt = xb_t[tj]
                pq = ps.tile([128, 512], F32, tag="ps", name=f"pq{tj}_{ct}")
                for c in range(CT):
                    nc.tensor.matmul(
                        pq,
                        wqk_sb[:, ct, c, :],
                        xt[:, c, :],
                        start=(c == 0),
                        stop=(c == CT - 1),
                    )
                nc.vector.tensor_copy(
                    out=qkt_sb[:, ct, tj * 512 : (tj + 1) * 512], in_=pq
                )

            def v_group(tj, tt):
                kt = tj * 4 + tt
                xt = xb_t[tj]
                pv = ps.tile([128, 512], F32, tag="ps", name=f"pv{kt}")
                for c in range(CT):
                    nc.tensor.matmul(
                        pv,
                        xt[:, c, tt * 128 : (tt + 1) * 128],
                        wv_sb[:, c, :],
                        start=(c == 0),
                        stop=(c == CT - 1),
                    )
                nc.vector.tensor_copy(
                    out=vv_sb[:, kt, :, 0:128],
                    in_=pv.rearrange("p (h d) -> p h d", h=HPC),
                )
                nc.vector.memset(vv_sb[:, kt, :, 128:129], 1.0)

            y_live = {}  # h -> y_tiles for the attention chunk in flight

            pt_live = {}  # (h, kt) -> exp'd P^T tile awaiting its AV phase

            def attn_s(j, h, sg):
                # S + exp + mask for segment sg (4 kt); AV runs later so the
                # ACT exp backlog is covered by interleaved GEMM groups
                if sg == 0:
                    pairs = [
                        yps.tile([128, 258], F32, tag="y", name=f"yp{h}_{j}_{q}")
                        for q in range(2)
                    ]
                    y_live[h] = [(pairs[qs // 2], (qs % 2) * 129) for qs in range(4)]
                for kt in range(4 * sg, 4 * sg + 4):
                    di = kt - 4 * j
                    lo = 128 * di if di > 0 else 0
                    ss = ps.tile([128, 512], F32, tag="ps", name=f"ss{h}{j}{kt}")
                    nc.tensor.matmul(
                        ss[:, lo:],
                        qkt_sb[:, 4 + h, kt * 128 : (kt + 1) * 128],
                        qkt_sb[:, h, j * 512 + lo : (j + 1) * 512],
                        start=True,
                        stop=True,
                    )
                    pt = ptp.tile([128, 512], F16, tag="pt", name=f"pt{h}{j}{kt}")
                    nc.scalar.activation(
                        out=pt[:, lo:], in_=ss[:, lo:], func=Exp, scale=SCALE
                    )
                    if di >= 0:
                        nc.vector.tensor_mul(
                            pt[:, lo : lo + 128],
                            pt[:, lo : lo + 128],
                            mask_sb[:, di, lo : lo + 128],
                        )
                    pt_live[(h, kt)] = pt

            def attn_av(j, h, sg, final=False):
                y_tiles = y_live[h]
                for kt in range(4 * sg, 4 * sg + 4):
                    di = kt - 4 * j
                    pt = pt_live.pop((h, kt))
                    for qs in range(max(0, di), 4):
                        # paired accumulators share a PSUM bank; start=True
                        # clears the whole bank, so only the off==0 group may
                        # use it (the clear also zeroes its bank-mate, which
                        # then accumulates from zero with start=False).
                        yp, off = y_tiles[qs]
                        nc.tensor.matmul(
                            yp[:, off : off + 129],
                            pt[:, qs * 128 : (qs + 1) * 128],
                            vv_sb[:, kt, h, :],
                            start=(kt == 0 and off == 0),
                            stop=(kt == 4 * j + qs),
                            skip_group_check=(off != 0),
                        )
                if final:
                    attn_head_end(j, h)

            def attn_head_end(j, h, on_pe=False):
                y_tiles = y_live.pop(h)
                for qs in range(4):
                    yp, off = y_tiles[qs]
                    r = rp.tile([128, 1], F32, tag="r", name=f"r{h}{j}{qs}")
                    nc.vector.reciprocal(r, yp[:, off + 128 : off + 129])
                    y16 = ysbp.tile([128, 128], F16, tag="y16", name=f"y16_{qs}")
                    nc.vector.tensor_scalar_mul(y16, yp[:, off : off + 128], r)
                    tglob = (j * 4 + qs) * 128
                    if on_pe:
                        # final head: PE transpose avoids the DMA-xbar latency
                        # right before the projection consumes yt
                        ytp = ps.tile([128, 128], F16, tag="ps", name=f"ytp{qs}")
                        nc.tensor.transpose(ytp, y16, ident_sb)
                        nc.vector.tensor_copy(
                            out=yt_sb[:, h, tglob : tglob + 128], in_=ytp
                        )
                    else:
                        nc.sync.dma_start_transpose(
                            out=yt_sb[:, h, tglob : tglob + 128], in_=y16
                        )

            ot_tiles = {}

            def proj_piece(tt, cc):
                # one quarter (512 out cols) of an output-projection tile,
                # copies on DVE; used to interleave with attention segments
                if cc == 0:
                    ot_tiles[tt] = ostp.tile([128, C], F16, tag="ot", name=f"ot{tt}")
                ot = ot_tiles[tt]
                po = ps.tile([128, 512], F32, tag="ps", name=f"po{tt}_{cc}")
                for hd in range(HPC):
                    nc.tensor.matmul(
                        po,
                        yt_sb[:, hd, tt * 128 : (tt + 1) * 128],
                        wp_sb[:, hd, cc * 512 : (cc + 1) * 512],
                        start=(hd == 0),
                        stop=(hd == HPC - 1),
                    )
                nc.vector.tensor_copy(out=ot[:, cc * 512 : (cc + 1) * 512], in_=po)
                if cc % 2 == 1:
                    nc.sync.dma_start(
                        out=out_d[
                            tt * 128 : (tt + 1) * 128,
                            (cc - 1) * 512 : (cc + 1) * 512,
                        ],
                        in_=ot[:, (cc - 1) * 512 : (cc + 1) * 512],
                    )
                if cc == 3:
                    ot_tiles.pop(tt)

            def proj_tile(tt, fine_store=False, dve_copies=False):
                ot = ostp.tile([128, C], F16, tag="ot", name=f"ot{tt}")
                for cc in range(4):
                    po = ps.tile([128, 512], F32, tag="ps", name=f"po{tt}_{cc}")
                    for hd in range(HPC):
                        nc.tensor.matmul(
                            po,
                            yt_sb[:, hd, tt * 128 : (tt + 1) * 128],
                            wp_sb[:, hd, cc * 512 : (cc + 1) * 512],
                            start=(hd == 0),
                            stop=(hd == HPC - 1),
                        )
                    if dve_copies or cc % 2 == 0:
                        nc.vector.tensor_copy(
                            out=ot[:, cc * 512 : (cc + 1) * 512], in_=po
                        )
                    else:
                        nc.scalar.activation(
                            out=ot[:, cc * 512 : (cc + 1) * 512], in_=po, func=Copy
                        )
                    if fine_store:
                        nc.sync.dma_start(
                            out=out_d[
                                tt * 128 : (tt + 1) * 128,
                                cc * 512 : (cc + 1) * 512,
                            ],
                            in_=ot[:, cc * 512 : (cc + 1) * 512],
                        )
                    elif cc % 2 == 1:
                        nc.sync.dma_start(
                            out=out_d[
                                tt * 128 : (tt + 1) * 128,
                                (cc - 1) * 512 : (cc + 1) * 512,
                            ],
                            in_=ot[:, (cc - 1) * 512 : (cc + 1) * 512],
                        )

            def chunk_groups(tj):
                for ct in (4, 5, 6, 7):
                    yield ("qkv", ct)
                for tt in range(4):
                    yield ("v", tt)
                for h in range(HPC):
                    yield ("qkv", h)

            # ---- chunk 0: pure QKV ----
            for kind, a in chunk_groups(0):
                (qkv_group if kind == "qkv" else v_group)(0, a)

            # ---- chunks 1..2: QKV(j) x attn(j-1) segments x proj(j-2) ----
            for j in range(1, NCH - 1):
                if j + 1 < NCH:
                    xb_t[j + 1] = xbp.tile(
                        [128, CT, 512], F16, tag="xb", name=f"xb{j + 1}"
                    )
                    nc.sync.dma_start(out=xb_t[j + 1], in_=xb_d[:, j + 1])
                aj = j - 1
                segs = [(h, s) for h in range(HPC) for s in range(aj + 1)]
                s_i = 0
                av_i = 0
                for gi, (kind, a) in enumerate(chunk_groups(j)):
                    (qkv_group if kind == "qkv" else v_group)(j, a)
                    while av_i < gi * len(segs) // 12:
                        h, s = segs[av_i]
                        attn_av(aj, h, s, final=(s == aj))
                        av_i += 1
                    while s_i < (gi + 1) * len(segs) // 12:
                        attn_s(aj, *segs[s_i])
                        s_i += 1
                while av_i < len(segs):
                    h, s = segs[av_i]
                    attn_av(aj, h, s, final=(s == aj))
                    av_i += 1
                if j >= 2:
                    for tt in range(4 * (j - 2), 4 * (j - 2) + 4):
                        proj_tile(tt)

            # ---- chunk 3: Q groups first, then K/V, x attn(2) segments.
            # attn(3) segs 0..2 only need K/V from chunks 0..2, so after the
            # Q groups the whole of attn(3) interleaves here, leaving only
            # pure projection work for the tail.
            j = NCH - 1
            aj = j - 1
            segs = [(h, s) for h in range(HPC) for s in range(aj + 1)]
            s_i = 0
            av_i = 0
            c3_groups = [("qkv", h) for h in range(HPC)]
            c3_groups += [("qkv", ct) for ct in (4, 5, 6, 7)]
            c3_groups += [("v", tt) for tt in range(4)]
            for gi, (kind, a) in enumerate(c3_groups):
                (qkv_group if kind == "qkv" else v_group)(j, a)
                while av_i < gi * len(segs) // 12:
                    h, s = segs[av_i]
                    attn_av(aj, h, s, final=(s == aj))
                    av_i += 1
                while s_i < (gi + 1) * len(segs) // 12:
                    attn_s(aj, *segs[s_i])
                    s_i += 1
            while av_i < len(segs):
                h, s = segs[av_i]
                attn_av(aj, h, s, final=(s == aj))
                av_i += 1
            # attn(3) heads with a proj(1) quarter-tile after every segment
            # as PE cover for the ACT exp backlog
            pieces = [
                (4 * (NCH - 3) + t2, cc) for t2 in range(4) for cc in range(4)
            ]
            pi = 0
            prev = None
            for h in range(HPC):
                for s in range(NCH):
                    attn_s(NCH - 1, h, s)
                    if pi < len(pieces):
                        proj_piece(*pieces[pi])
                        pi += 1
                    if prev is not None:
                        ph, psg = prev
                        attn_av(NCH - 1, ph, psg, final=(psg == NCH - 1))
                    prev = (h, s)
            attn_av(NCH - 1, prev[0], prev[1], final=True)
            # pure projection tail: proj(2) then proj(3)
            for tt in range(4 * (NCH - 2), 4 * NCH):
                proj_tile(tt, fine_store=(tt == 4 * NCH - 1))

    nc.compile()
    return nc


# revision 28
# speedup vs baseline: 1.0146x; 1.0048x over previous
"""Causal self-attention (B=2, T=2048, C=2048, H=16) on 8 TRN2 NeuronCores.

Sharding: data-parallel over batch (2) x tensor-parallel over heads (4 heads
per core). Each core computes, for its batch element b and head group g:
  QKV projection for its heads' columns, causal attention for its 4 heads,
  and a partial output projection (row-sharded W_proj). The host sums the
  4 partial projections per batch element.

Key performance structure:
  - All inputs host-prepacked into partition-major layouts so every DMA
    moves >=2KB contiguous per partition line, issued in first-use order
    on the sync HWDGE ring. The first K-head weight group and x chunk are
    split into half-size DMAs so the first projection chain starts after
    ~1.25MB; its first ~13 cold matmuls double as the PE HAM clock-gate
    warm-up (2.4 GHz reached ~6us in).
  - Software-pipelined chunk schedule: chunk j's QKV projection groups are
    interleaved at ~4us granularity with attention SEGMENTS (4 key-tiles)
    of chunk j-1 and the output projection of chunk j-2. Each segment is
    further split into an S phase (S^T matmuls + exp + mask) and an AV
    phase emitted one GEMM group later, so the ACT exp latency is hidden
    under a full ~4.3us projection group. Every cross-engine dependency
    (PSUM->DVE qkt copy, ACT exp backlog, DMA transpose) is at least one
    GEMM group old by the time the PE consumes it, so the PE never waits.
    Chunk 3 emits its Q groups FIRST so all of chunk 3's attention (whose
    kt<=11 tiles only need K/V from chunks 0..2) interleaves there too,
    with quarter-tiles of the chunk-1 output projection as PE cover for
    the ACT exp backlog; the tail is pure projection streaming.
  - The softmax accumulators for 4 q-subtiles live as 2 PSUM bank pairs;
    only the first accumulator of a pair issues start=True (the bank clear
    also zeroes its bank-mate, which accumulates from zero).
  - The y^T transpose needed by the output projection runs on the DMA xbar
    (dma_start_transpose), not the PE.

Per-core device layouts (fp16 compute / fp32 PSUM accumulation):
  xb   [128, 4, 16, 512]  x^T tiles, chunk-major: [p, tj, c, t]
  wqk  [128, 8, 16, 128]  [p, coltile, c, n]; coltiles 0..3 Q heads, 4..7 K
  wv   [128, 16, 512]     [p, c, (h d)]
  wp   [128, 4, 2048]     [p, h, c]  W_proj rows for this head group
  out  [T, C] fp16 partial projection output

Attention per (head, 512-wide q-chunk): S^T = K_kt^T.T @ Q^T per key tile,
P^T = exp(scale*S^T) (ACT), diagonal masks on DVE, Y[q, d+1] += P^T.T @
[V | ones] accumulated in PSUM (the ones column gives the softmax
denominator), y = Y[:, :d] * (1/Y[:, d]) on DVE, then DMA-transpose into
yt[d, h, t] for the projection.
"""

import os

import numpy as np

N_HEAD = 16
N_EMBD = 2048
B = 2
T = 2048
C = N_EMBD
D = C // N_HEAD  # 128
HPC = N_HEAD // 4  # heads per core = 4
N_CORES = 8
CT = C // 128  # 16 contraction tiles
TT = T // 128  # 16 t tiles
NCH = T // 512  # 4 chunks of 512

LAST_EXEC_NS = None

_CACHE = {}


def _build_nc():
    import concourse.bass as bass  # noqa: F401
    import concourse.tile as tile
    from concourse import bacc, mybir

    F32 = mybir.dt.float32
    F16 = mybir.dt.float16
    Exp = mybir.ActivationFunctionType.Exp
    Copy = mybir.ActivationFunctionType.Copy
    SCALE = 1.0 / float(np.sqrt(D))

    nc = bacc.Bacc("TRN2", target_bir_lowering=False, num_devices=N_CORES)

    xb_d = nc.dram_tensor("xb", [128, NCH, CT, 512], F16, kind="ExternalInput")
    wqk_d = nc.dram_tensor("wqk", [128, 8, CT, 128], F16, kind="ExternalInput")
    wv_d = nc.dram_tensor("wv", [128, CT, 512], F16, kind="ExternalInput")
    wp_d = nc.dram_tensor("wp", [128, HPC, C], F16, kind="ExternalInput")
    out_d = nc.dram_tensor("out_part", [T, C], F16, kind="ExternalOutput")

    # Diagonal causal masks, partition-major: [128 k, diag idx, 512 q].
    kk = np.arange(128)[:, None]
    qq = np.arange(512)[None, :]
    masks = np.stack(
        [(qq >= (128 * i + kk)).astype(np.float16) for i in range(4)], axis=1
    )  # [128, 4, 512]
    masks_d = nc.inline_tensor(np.ascontiguousarray(masks), name="diagmasks")
    ident_d = nc.inline_tensor(np.eye(128, dtype=np.float16), name="ident128")

    with tile.TileContext(nc) as tc:
        with (
            tc.tile_pool(name="singles", bufs=1) as singles,
            tc.tile_pool(name="xbp", bufs=3) as xbp,
            tc.tile_pool(name="ptp", bufs=10) as ptp,
            tc.tile_pool(name="ysb", bufs=4) as ysbp,
            tc.tile_pool(name="rp", bufs=4) as rp,
            tc.tile_pool(name="ost", bufs=3) as ostp,
            tc.tile_pool(name="ps", bufs=6, space="PSUM") as ps,
            tc.tile_pool(name="yps", bufs=2, space="PSUM") as yps,
        ):
            # ---- Input DMAs, issued in first-use order on the sync ring ----
            wqk_sb = singles.tile([128, 8, CT, 128], F16, name="wqk_sb")
            wv_sb = singles.tile([128, CT, 512], F16, name="wv_sb")
            wp_sb = singles.tile([128, HPC, C], F16, name="wp_sb")
            mask_sb = singles.tile([128, 4, 512], F16, name="mask_sb")
            xb_t = [None] * NCH

            xb_t[0] = xbp.tile([128, CT, 512], F16, tag="xb", name="xb0")
            # identity loads first (32KB): a spin of N=128 matmuls on it fills
            # the otherwise-idle DMA window and holds the HAM clock-gate at
            # 8/8 so the first real chain runs at 2.4 GHz from its first MM
            ident_sb = singles.tile([128, 128], F16, name="ident_sb")
            nc.sync.dma_start(out=ident_sb, in_=ident_d[:, :])
            # first K group + x chunk 0, split in halves: the first 8 matmuls
            # of the first chain start after ~1.25MB instead of 2.5MB
            nc.sync.dma_start(out=wqk_sb[:, 4, 0:8], in_=wqk_d[:, 4, 0:8])
            nc.sync.dma_start(out=xb_t[0][:, 0:8], in_=xb_d[:, 0, 0:8])
            nc.sync.dma_start(out=wqk_sb[:, 4, 8:16], in_=wqk_d[:, 4, 8:16])
            nc.sync.dma_start(out=xb_t[0][:, 8:16], in_=xb_d[:, 0, 8:16])
            for ct in (5, 6, 7):
                nc.sync.dma_start(out=wqk_sb[:, ct], in_=wqk_d[:, ct])
            nc.sync.dma_start(out=wv_sb, in_=wv_d[:, :])
            for ct in (0, 1, 2, 3):
                nc.sync.dma_start(out=wqk_sb[:, ct], in_=wqk_d[:, ct])
            nc.sync.dma_start(out=mask_sb, in_=masks_d[:, :, :])
            nc.sync.dma_start(out=wp_sb, in_=wp_d[:, :])
            xb_t[1] = xbp.tile([128, CT, 512], F16, tag="xb", name="xb1")
            nc.sync.dma_start(out=xb_t[1], in_=xb_d[:, 1])

            for i in range(80):
                wsp = ps.tile([128, 128], F32, tag="ps", name=f"spin{i}")
                nc.tensor.matmul(wsp, ident_sb, ident_sb, start=True, stop=True)

            # qkt: [d, coltile, t]; coltiles 0..3 = Q heads, 4..7 = K heads
            qkt_sb = singles.tile([128, 8, T], F16)
            # v with a ones column per (kt, head): [kt-tile, head, 129]
            vv_sb = singles.tile([128, TT, HPC, 129], F16)
            # y transposed: [d, head, t]
            yt_sb = singles.tile([128, HPC, T], F16)

            def qkv_group(tj, ct):
                # projection group for coltile ct of chunk tj (N=512, 16 MMs)
                xt = xb_t[tj]
                pq = ps.tile([128, 512], F32, tag="ps", name=f"pq{tj}_{ct}")
                for c in range(CT):
                    nc.tensor.matmul(
                        pq,
                        wqk_sb[:, ct, c, :],
                        xt[:, c, :],
                        start=(c == 0),
                        stop=(c == CT - 1),
                    )
                nc.vector.tensor_copy(
                    out=qkt_sb[:, ct, tj * 512 : (tj + 1) * 512], in_=pq
                )

            def v_group(tj, tt):
                kt = tj * 4 + tt
                xt = xb_t[tj]
                pv = ps.tile([128, 512], F32, tag="ps", name=f"pv{kt}")
                for c in range(CT):
                    nc.tensor.matmul(
                        pv,
                        xt[:, c, tt * 128 : (tt + 1) * 128],
                        wv_sb[:, c, :],
                        start=(c == 0),
                        stop=(c == CT - 1),
                    )
                nc.vector.tensor_copy(
                    out=vv_sb[:, kt, :, 0:128],
                    in_=pv.rearrange("p (h d) -> p h d", h=HPC),
                )
                nc.vector.memset(vv_sb[:, kt, :, 128:129], 1.0)

            y_live = {}  # h -> y_tiles for the attention chunk in flight

            pt_live = {}  # (h, kt) -> exp'd P^T tile awaiting its AV phase

            def attn_s(j, h, sg):
                # S + exp + mask for segment sg (4 kt); AV runs later so the
                # ACT exp backlog is covered by interleaved GEMM groups
                if sg == 0:
                    pairs = [
                        yps.tile([128, 258], F32, tag="y", name=f"yp{h}_{j}_{q}")
                        for q in range(2)
                    ]
                    y_live[h] = [(pairs[qs // 2], (qs % 2) * 129) for qs in range(4)]
                for kt in range(4 * sg, 4 * sg + 4):
                    di = kt - 4 * j
                    lo = 128 * di if di > 0 else 0
                    ss = ps.tile([128, 512], F32, tag="ps", name=f"ss{h}{j}{kt}")
                    nc.tensor.matmul(
                        ss[:, lo:],
                        qkt_sb[:, 4 + h, kt * 128 : (kt + 1) * 128],
                        qkt_sb[:, h, j * 512 + lo : (j + 1) * 512],
                        start=True,
                        stop=True,
                    )
                    pt = ptp.tile([128, 512], F16, tag="pt", name=f"pt{h}{j}{kt}")
                    nc.scalar.activation(
                        out=pt[:, lo:], in_=ss[:, lo:], func=Exp, scale=SCALE
                    )
                    if di >= 0:
                        nc.vector.tensor_mul(
                            pt[:, lo : lo + 128],
                            pt[:, lo : lo + 128],
                            mask_sb[:, di, lo : lo + 128],
                        )
                    pt_live[(h, kt)] = pt

            def attn_av(j, h, sg, final=False):
                y_tiles = y_live[h]
                for kt in range(4 * sg, 4 * sg + 4):
                    di = kt - 4 * j
                    pt = pt_live.pop((h, kt))
                    for qs in range(max(0, di), 4):
                        # paired accumulators share a PSUM bank; start=True
                        # clears the whole bank, so only the off==0 group may
                        # use it (the clear also zeroes its bank-mate, which
                        # then accumulates from zero with start=False).
                        yp, off = y_tiles[qs]
                        nc.tensor.matmul(
                            yp[:, off : off + 129],
                            pt[:, qs * 128 : (qs + 1) * 128],
                            vv_sb[:, kt, h, :],
                            start=(kt == 0 and off == 0),
                            stop=(kt == 4 * j + qs),
                            skip_group_check=(off != 0),
                        )
                if final:
                    attn_head_end(j, h)

            def attn_head_end(j, h, on_pe=False):
                y_tiles = y_live.pop(h)
                for qs in range(4):
                    yp, off = y_tiles[qs]
                    r = rp.tile([128, 1], F32, tag="r", name=f"r{h}{j}{qs}")
                    nc.vector.reciprocal(r, yp[:, off + 128 : off + 129])
                    y16 = ysbp.tile([128, 128], F16, tag="y16", name=f"y16_{qs}")
                    nc.vector.tensor_scalar_mul(y16, yp[:, off : off + 128], r)
                    tglob = (j * 4 + qs) * 128
                    if on_pe:
                        # final head: PE transpose avoids the DMA-xbar latency
                        # right before the projection consumes yt
                        ytp = ps.tile([128, 128], F16, tag="ps", name=f"ytp{qs}")
                        nc.tensor.transpose(ytp, y16, ident_sb)
                        nc.vector.tensor_copy(
                            out=yt_sb[:, h, tglob : tglob + 128], in_=ytp
                        )
                    else:
                        nc.sync.dma_start_transpose(
                            out=yt_sb[:, h, tglob : tglob + 128], in_=y16
                        )

            ot_tiles = {}

            def proj_piece(tt, cc):
                # one quarter (512 out cols) of an output-projection tile,
                # copies on DVE; used to interleave with attention segments
                if cc == 0:
                    ot_tiles[tt] = ostp.tile([128, C], F16, tag="ot", name=f"ot{tt}")
                ot = ot_tiles[tt]
                po = ps.tile([128, 512], F32, tag="ps", name=f"po{tt}_{cc}")
                for hd in range(HPC):
                    nc.tensor.matmul(
                        po,
                        yt_sb[:, hd, tt * 128 : (tt + 1) * 128],
                        wp_sb[:, hd, cc * 512 : (cc + 1) * 512],
                        start=(hd == 0),
                        stop=(hd == HPC - 1),
                    )
                nc.vector.tensor_copy(out=ot[:, cc * 512 : (cc + 1) * 512], in_=po)
                if cc % 2 == 1:
                    nc.sync.dma_start(
                        out=out_d[
                            tt * 128 : (tt + 1) * 128,
                            (cc - 1) * 512 : (cc + 1) * 512,
                        ],
                        in_=ot[:, (cc - 1) * 512 : (cc + 1) * 512],
                    )
                if cc == 3:
                    ot_tiles.pop(tt)

            def proj_tile(tt, fine_store=False, dve_copies=False):
                ot = ostp.tile([128, C], F16, tag="ot", name=f"ot{tt}")
                for cc in range(4):
                    po = ps.tile([128, 512], F32, tag="ps", name=f"po{tt}_{cc}")
                    for hd in range(HPC):
                        nc.tensor.matmul(
                            po,
                            yt_sb[:, hd, tt * 128 : (tt + 1) * 128],
                            wp_sb[:, hd, cc * 512 : (cc + 1) * 512],
                            start=(hd == 0),
                            stop=(hd == HPC - 1),
                        )
                    if fine_store and cc == 3:
                        # final quarter of the kernel: halve the copy latency
                        # by splitting it across DVE and ACT, then two stores
                        nc.vector.tensor_copy(
                            out=ot[:, cc * 512 : cc * 512 + 256], in_=po[:, 0:256]
                        )
                        nc.scalar.activation(
                            out=ot[:, cc * 512 + 256 : (cc + 1) * 512],
                            in_=po[:, 256:512],
                            func=Copy,
                        )
                        nc.sync.dma_start(
                            out=out_d[
                                tt * 128 : (tt + 1) * 128, cc * 512 : cc * 512 + 256
                            ],
                            in_=ot[:, cc * 512 : cc * 512 + 256],
                        )
                        nc.sync.dma_start(
                            out=out_d[
                                tt * 128 : (tt + 1) * 128,
                                cc * 512 + 256 : (cc + 1) * 512,
                            ],
                            in_=ot[:, cc * 512 + 256 : (cc + 1) * 512],
                        )
                        continue
                    if dve_copies or cc % 2 == 0:
                        nc.vector.tensor_copy(
                            out=ot[:, cc * 512 : (cc + 1) * 512], in_=po
                        )
                    else:
                        nc.scalar.activation(
                            out=ot[:, cc * 512 : (cc + 1) * 512], in_=po, func=Copy
                        )
                    if fine_store:
                        nc.sync.dma_start(
                            out=out_d[
                                tt * 128 : (tt + 1) * 128,
                                cc * 512 : (cc + 1) * 512,
                            ],
                            in_=ot[:, cc * 512 : (cc + 1) * 512],
                        )
                        continue
                    elif cc % 2 == 1:
                        nc.sync.dma_start(
                            out=out_d[
                                tt * 128 : (tt + 1) * 128,
                                (cc - 1) * 512 : (cc + 1) * 512,
                            ],
                            in_=ot[:, (cc - 1) * 512 : (cc + 1) * 512],
                        )

            def chunk_groups(tj):
                for ct in (4, 5, 6, 7):
                    yield ("qkv", ct)
                for tt in range(4):
                    yield ("v", tt)
                for h in range(HPC):
                    yield ("qkv", h)

            # ---- chunk 0: pure QKV ----
            for kind, a in chunk_groups(0):
                (qkv_group if kind == "qkv" else v_group)(0, a)

            # ---- chunks 1..2: QKV(j) x attn(j-1) segments x proj(j-2) ----
            for j in range(1, NCH - 1):
                if j + 1 < NCH:
                    xb_t[j + 1] = xbp.tile(
                        [128, CT, 512], F16, tag="xb", name=f"xb{j + 1}"
                    )
                    nc.sync.dma_start(out=xb_t[j + 1], in_=xb_d[:, j + 1])
                aj = j - 1
                segs = [(h, s) for h in range(HPC) for s in range(aj + 1)]
                s_i = 0
                av_i = 0
                for gi, (kind, a) in enumerate(chunk_groups(j)):
                    (qkv_group if kind == "qkv" else v_group)(j, a)
                    while av_i < gi * len(segs) // 12:
                        h, s = segs[av_i]
                        attn_av(aj, h, s, final=(s == aj))
                        av_i += 1
                    while s_i < (gi + 1) * len(segs) // 12:
                        attn_s(aj, *segs[s_i])
                        s_i += 1
                while av_i < len(segs):
                    h, s = segs[av_i]
                    attn_av(aj, h, s, final=(s == aj))
                    av_i += 1
                if j >= 2:
                    for tt in range(4 * (j - 2), 4 * (j - 2) + 4):
                        proj_tile(tt)

            # ---- chunk 3: Q groups first, then K/V, x attn(2) segments.
            # attn(3) segs 0..2 only need K/V from chunks 0..2, so after the
            # Q groups the whole of attn(3) interleaves here, leaving only
            # pure projection work for the tail.
            j = NCH - 1
            aj = j - 1
            segs = [(h, s) for h in range(HPC) for s in range(aj + 1)]
            s_i = 0
            av_i = 0
            c3_groups = [("qkv", h) for h in range(HPC)]
            c3_groups += [("qkv", ct) for ct in (4, 5, 6, 7)]
            c3_groups += [("v", tt) for tt in range(4)]
            for gi, (kind, a) in enumerate(c3_groups):
                (qkv_group if kind == "qkv" else v_group)(j, a)
                while av_i < gi * len(segs) // 12:
                    h, s = segs[av_i]
                    attn_av(aj, h, s, final=(s == aj))
                    av_i += 1
                while s_i < (gi + 1) * len(segs) // 12:
                    attn_s(aj, *segs[s_i])
                    s_i += 1
            while av_i < len(segs):
                h, s = segs[av_i]
                attn_av(aj, h, s, final=(s == aj))
                av_i += 1
            # attn(3) heads with a proj(1) quarter-tile after every segment
            # as PE cover for the ACT exp backlog
            pieces = [
                (4 * (NCH - 3) + t2, cc) for t2 in range(4) for cc in range(4)
            ]
            pi = 0
            prev = None
            for h in range(HPC):
                for s in range(NCH):
                    attn_s(NCH - 1, h, s)
                    if pi < len(pieces):
                        proj_piece(*pieces[pi])
                        pi += 1
                    if prev is not None:
                        ph, psg = prev
                        attn_av(NCH - 1, ph, psg, final=(psg == NCH - 1))
                    prev = (h, s)
            attn_av(NCH - 1, prev[0], prev[1], final=True)
            # pure projection tail: proj(2) then proj(3)
            for tt in range(4 * (NCH - 2), 4 * NCH):
                proj_tile(tt, fine_store=(tt == 4 * NCH - 1))

    nc.compile()
    return nc


# revision 29
# speedup vs baseline: 1.0179x; 1.0032x over previous
"""Causal self-attention (B=2, T=2048, C=2048, H=16) on 8 TRN2 NeuronCores.

Sharding: data-parallel over batch (2) x tensor-parallel over heads (4 heads
per core). Each core computes, for its batch element b and head group g:
  QKV projection for its heads' columns, causal attention for its 4 heads,
  and a partial output projection (row-sharded W_proj). The host sums the
  4 partial projections per batch element.

Key performance structure:
  - All inputs host-prepacked into partition-major layouts so every DMA
    moves >=2KB contiguous per partition line, issued in first-use order
    on the sync HWDGE ring. The first K-head weight group and x chunk are
    split into half-size DMAs so the first projection chain starts after
    ~1.25MB; its first ~13 cold matmuls double as the PE HAM clock-gate
    warm-up (2.4 GHz reached ~6us in).
  - Software-pipelined chunk schedule: chunk j's QKV projection groups are
    interleaved at ~4us granularity with attention SEGMENTS (4 key-tiles)
    of chunk j-1 and the output projection of chunk j-2. Each segment is
    further split into an S phase (S^T matmuls + exp + mask) and an AV
    phase emitted one GEMM group later, so the ACT exp latency is hidden
    under a full ~4.3us projection group. Every cross-engine dependency
    (PSUM->DVE qkt copy, ACT exp backlog, DMA transpose) is at least one
    GEMM group old by the time the PE consumes it, so the PE never waits.
    Chunk 3 emits its Q groups FIRST so all of chunk 3's attention (whose
    kt<=11 tiles only need K/V from chunks 0..2) interleaves there too,
    with quarter-tiles of the chunk-1 output projection as PE cover for
    the ACT exp backlog; the tail is pure projection streaming.
  - The softmax accumulators for 4 q-subtiles live as 2 PSUM bank pairs;
    only the first accumulator of a pair issues start=True (the bank clear
    also zeroes its bank-mate, which accumulates from zero).
  - The y^T transpose needed by the output projection runs on the DMA xbar
    (dma_start_transpose), not the PE.

Per-core device layouts (fp16 compute / fp32 PSUM accumulation):
  xb   [128, 4, 16, 512]  x^T tiles, chunk-major: [p, tj, c, t]
  wqk  [128, 8, 16, 128]  [p, coltile, c, n]; coltiles 0..3 Q heads, 4..7 K
  wv   [128, 16, 512]     [p, c, (h d)]
  wp   [128, 4, 2048]     [p, h, c]  W_proj rows for this head group
  out  [T, C] fp16 partial projection output

Attention per (head, 512-wide q-chunk): S^T = K_kt^T.T @ Q^T per key tile,
P^T = exp(scale*S^T) (ACT), diagonal masks on DVE, Y[q, d+1] += P^T.T @
[V | ones] accumulated in PSUM (the ones column gives the softmax
denominator), y = Y[:, :d] * (1/Y[:, d]) on DVE, then DMA-transpose into
yt[d, h, t] for the projection.
"""

import os

import numpy as np

N_HEAD = 16
N_EMBD = 2048
B = 2
T = 2048
C = N_EMBD
D = C // N_HEAD  # 128
HPC = N_HEAD // 4  # heads per core = 4
N_CORES = 8
CT = C // 128  # 16 contraction tiles
TT = T // 128  # 16 t tiles
NCH = T // 512  # 4 chunks of 512

LAST_EXEC_NS = None

_CACHE = {}


def _build_nc():
    import concourse.bass as bass  # noqa: F401
    import concourse.tile as tile
    from concourse import bacc, mybir

    F32 = mybir.dt.float32
    F16 = mybir.dt.float16
    Exp = mybir.ActivationFunctionType.Exp
    Copy = mybir.ActivationFunctionType.Copy
    SCALE = 1.0 / float(np.sqrt(D))

    nc = bacc.Bacc("TRN2", target_bir_lowering=False, num_devices=N_CORES)

    xb_d = nc.dram_tensor("xb", [128, NCH, CT, 512], F16, kind="ExternalInput")
    wqk_d = nc.dram_tensor("wqk", [128, 8, CT, 128], F16, kind="ExternalInput")
    wv_d = nc.dram_tensor("wv", [128, CT, 512], F16, kind="ExternalInput")
    wp_d = nc.dram_tensor("wp", [128, HPC, C], F16, kind="ExternalInput")
    out_d = nc.dram_tensor("out_part", [T, C], F16, kind="ExternalOutput")

    # Diagonal causal masks, partition-major: [128 k, diag idx, 512 q].
    kk = np.arange(128)[:, None]
    qq = np.arange(512)[None, :]
    masks = np.stack(
        [(qq >= (128 * i + kk)).astype(np.float16) for i in range(4)], axis=1
    )  # [128, 4, 512]
    masks_d = nc.inline_tensor(np.ascontiguousarray(masks), name="diagmasks")
    ident_d = nc.inline_tensor(np.eye(128, dtype=np.float16), name="ident128")

    with tile.TileContext(nc) as tc:
        with (
            tc.tile_pool(name="singles", bufs=1) as singles,
            tc.tile_pool(name="xbp", bufs=3) as xbp,
            tc.tile_pool(name="ptp", bufs=10) as ptp,
            tc.tile_pool(name="ysb", bufs=4) as ysbp,
            tc.tile_pool(name="rp", bufs=4) as rp,
            tc.tile_pool(name="ost", bufs=3) as ostp,
            tc.tile_pool(name="ps", bufs=6, space="PSUM") as ps,
            tc.tile_pool(name="yps", bufs=2, space="PSUM") as yps,
        ):
            # ---- Input DMAs, issued in first-use order on the sync ring ----
            wqk_sb = singles.tile([128, 8, CT, 128], F16, name="wqk_sb")
            wv_sb = singles.tile([128, CT, 512], F16, name="wv_sb")
            wp_sb = singles.tile([128, HPC, C], F16, name="wp_sb")
            mask_sb = singles.tile([128, 4, 512], F16, name="mask_sb")
            xb_t = [None] * NCH

            xb_t[0] = xbp.tile([128, CT, 512], F16, tag="xb", name="xb0")
            # first K group + x chunk 0, split in halves: the first 8 matmuls
            # of the first chain start after ~1.25MB instead of 2.5MB (these
            # cold matmuls also warm the PE HAM clock-gate)
            nc.sync.dma_start(out=wqk_sb[:, 4, 0:8], in_=wqk_d[:, 4, 0:8])
            nc.sync.dma_start(out=xb_t[0][:, 0:8], in_=xb_d[:, 0, 0:8])
            nc.sync.dma_start(out=wqk_sb[:, 4, 8:16], in_=wqk_d[:, 4, 8:16])
            nc.sync.dma_start(out=xb_t[0][:, 8:16], in_=xb_d[:, 0, 8:16])
            for ct in (5, 6, 7):
                nc.sync.dma_start(out=wqk_sb[:, ct], in_=wqk_d[:, ct])
            nc.sync.dma_start(out=wv_sb, in_=wv_d[:, :])
            for ct in (0, 1, 2, 3):
                nc.sync.dma_start(out=wqk_sb[:, ct], in_=wqk_d[:, ct])
            nc.sync.dma_start(out=mask_sb, in_=masks_d[:, :, :])
            ident_sb = singles.tile([128, 128], F16, name="ident_sb")
            nc.sync.dma_start(out=ident_sb, in_=ident_d[:, :])
            nc.sync.dma_start(out=wp_sb, in_=wp_d[:, :])
            xb_t[1] = xbp.tile([128, CT, 512], F16, tag="xb", name="xb1")
            nc.sync.dma_start(out=xb_t[1], in_=xb_d[:, 1])

            # qkt: [d, coltile, t]; coltiles 0..3 = Q heads, 4..7 = K heads
            qkt_sb = singles.tile([128, 8, T], F16)
            # v with a ones column per (kt, head): [kt-tile, head, 129]
            vv_sb = singles.tile([128, TT, HPC, 129], F16)
            # y transposed: [d, head, t]
            yt_sb = singles.tile([128, HPC, T], F16)

            def qkv_group(tj, ct):
                # projection group for coltile ct of chunk tj (N=512, 16 MMs)
                xt = xb_t[tj]
                pq = ps.tile([128, 512], F32, tag="ps", name=f"pq{tj}_{ct}")
                for c in range(CT):
                    nc.tensor.matmul(
                        pq,
                        wqk_sb[:, ct, c, :],
                        xt[:, c, :],
                        start=(c == 0),
                        stop=(c == CT - 1),
                    )
                nc.vector.tensor_copy(
                    out=qkt_sb[:, ct, tj * 512 : (tj + 1) * 512], in_=pq
                )

            def v_group(tj, tt):
                kt = tj * 4 + tt
                xt = xb_t[tj]
                pv = ps.tile([128, 512], F32, tag="ps", name=f"pv{kt}")
                for c in range(CT):
                    nc.tensor.matmul(
                        pv,
                        xt[:, c, tt * 128 : (tt + 1) * 128],
                        wv_sb[:, c, :],
                        start=(c == 0),
                        stop=(c == CT - 1),
                    )
                nc.vector.tensor_copy(
                    out=vv_sb[:, kt, :, 0:128],
                    in_=pv.rearrange("p (h d) -> p h d", h=HPC),
                )
                nc.vector.memset(vv_sb[:, kt, :, 128:129], 1.0)

            y_live = {}  # h -> y_tiles for the attention chunk in flight

            pt_live = {}  # (h, kt) -> exp'd P^T tile awaiting its AV phase

            def attn_s(j, h, sg):
                # S + exp + mask for segment sg (4 kt); AV runs later so the
                # ACT exp backlog is covered by interleaved GEMM groups
                if sg == 0:
                    pairs = [
                        yps.tile([128, 258], F32, tag="y", name=f"yp{h}_{j}_{q}")
                        for q in range(2)
                    ]
                    y_live[h] = [(pairs[qs // 2], (qs % 2) * 129) for qs in range(4)]
                for kt in range(4 * sg, 4 * sg + 4):
                    di = kt - 4 * j
                    lo = 128 * di if di > 0 else 0
                    ss = ps.tile([128, 512], F32, tag="ps", name=f"ss{h}{j}{kt}")
                    nc.tensor.matmul(
                        ss[:, lo:],
                        qkt_sb[:, 4 + h, kt * 128 : (kt + 1) * 128],
                        qkt_sb[:, h, j * 512 + lo : (j + 1) * 512],
                        start=True,
                        stop=True,
                    )
                    pt = ptp.tile([128, 512], F16, tag="pt", name=f"pt{h}{j}{kt}")
                    nc.scalar.activation(
                        out=pt[:, lo:], in_=ss[:, lo:], func=Exp, scale=SCALE
                    )
                    if di >= 0:
                        nc.vector.tensor_mul(
                            pt[:, lo : lo + 128],
                            pt[:, lo : lo + 128],
                            mask_sb[:, di, lo : lo + 128],
                        )
                    pt_live[(h, kt)] = pt

            def attn_av(j, h, sg, final=False):
                y_tiles = y_live[h]
                for kt in range(4 * sg, 4 * sg + 4):
                    di = kt - 4 * j
                    pt = pt_live.pop((h, kt))
                    for qs in range(max(0, di), 4):
                        # paired accumulators share a PSUM bank; start=True
                        # clears the whole bank, so only the off==0 group may
                        # use it (the clear also zeroes its bank-mate, which
                        # then accumulates from zero with start=False).
                        yp, off = y_tiles[qs]
                        nc.tensor.matmul(
                            yp[:, off : off + 129],
                            pt[:, qs * 128 : (qs + 1) * 128],
                            vv_sb[:, kt, h, :],
                            start=(kt == 0 and off == 0),
                            stop=(kt == 4 * j + qs),
                            skip_group_check=(off != 0),
                        )
                if final:
                    attn_head_end(j, h)

            def attn_head_end(j, h, on_pe=False):
                y_tiles = y_live.pop(h)
                for qs in range(4):
                    yp, off = y_tiles[qs]
                    r = rp.tile([128, 1], F32, tag="r", name=f"r{h}{j}{qs}")
                    nc.vector.reciprocal(r, yp[:, off + 128 : off + 129])
                    y16 = ysbp.tile([128, 128], F16, tag="y16", name=f"y16_{qs}")
                    nc.vector.tensor_scalar_mul(y16, yp[:, off : off + 128], r)
                    tglob = (j * 4 + qs) * 128
                    if on_pe:
                        # final head: PE transpose avoids the DMA-xbar latency
                        # right before the projection consumes yt
                        ytp = ps.tile([128, 128], F16, tag="ps", name=f"ytp{qs}")
                        nc.tensor.transpose(ytp, y16, ident_sb)
                        nc.vector.tensor_copy(
                            out=yt_sb[:, h, tglob : tglob + 128], in_=ytp
                        )
                    else:
                        nc.sync.dma_start_transpose(
                            out=yt_sb[:, h, tglob : tglob + 128], in_=y16
                        )

            ot_tiles = {}

            def proj_piece(tt, cc):
                # one quarter (512 out cols) of an output-projection tile,
                # copies on DVE; used to interleave with attention segments
                if cc == 0:
                    ot_tiles[tt] = ostp.tile([128, C], F16, tag="ot", name=f"ot{tt}")
                ot = ot_tiles[tt]
                po = ps.tile([128, 512], F32, tag="ps", name=f"po{tt}_{cc}")
                for hd in range(HPC):
                    nc.tensor.matmul(
                        po,
                        yt_sb[:, hd, tt * 128 : (tt + 1) * 128],
                        wp_sb[:, hd, cc * 512 : (cc + 1) * 512],
                        start=(hd == 0),
                        stop=(hd == HPC - 1),
                    )
                nc.vector.tensor_copy(out=ot[:, cc * 512 : (cc + 1) * 512], in_=po)
                if cc % 2 == 1:
                    nc.sync.dma_start(
                        out=out_d[
                            tt * 128 : (tt + 1) * 128,
                            (cc - 1) * 512 : (cc + 1) * 512,
                        ],
                        in_=ot[:, (cc - 1) * 512 : (cc + 1) * 512],
                    )
                if cc == 3:
                    ot_tiles.pop(tt)

            def proj_tile(tt, fine_store=False, dve_copies=False):
                ot = ostp.tile([128, C], F16, tag="ot", name=f"ot{tt}")
                for cc in range(4):
                    po = ps.tile([128, 512], F32, tag="ps", name=f"po{tt}_{cc}")
                    for hd in range(HPC):
                        nc.tensor.matmul(
                            po,
                            yt_sb[:, hd, tt * 128 : (tt + 1) * 128],
                            wp_sb[:, hd, cc * 512 : (cc + 1) * 512],
                            start=(hd == 0),
                            stop=(hd == HPC - 1),
                        )
                    if dve_copies or cc % 2 == 0:
                        nc.vector.tensor_copy(
                            out=ot[:, cc * 512 : (cc + 1) * 512], in_=po
                        )
                    else:
                        nc.scalar.activation(
                            out=ot[:, cc * 512 : (cc + 1) * 512], in_=po, func=Copy
                        )
                    if fine_store:
                        nc.sync.dma_start(
                            out=out_d[
                                tt * 128 : (tt + 1) * 128,
                                cc * 512 : (cc + 1) * 512,
                            ],
                            in_=ot[:, cc * 512 : (cc + 1) * 512],
                        )
                    elif cc % 2 == 1:
                        nc.sync.dma_start(
                            out=out_d[
                                tt * 128 : (tt + 1) * 128,
                                (cc - 1) * 512 : (cc + 1) * 512,
                            ],
                            in_=ot[:, (cc - 1) * 512 : (cc + 1) * 512],
                        )

            def chunk_groups(tj):
                for ct in (4, 5, 6, 7):
                    yield ("qkv", ct)
                for tt in range(4):
                    yield ("v", tt)
                for h in range(HPC):
                    yield ("qkv", h)

            # ---- chunk 0: pure QKV ----
            for kind, a in chunk_groups(0):
                (qkv_group if kind == "qkv" else v_group)(0, a)

            # ---- chunks 1..2: QKV(j) x attn(j-1) segments x proj(j-2) ----
            for j in range(1, NCH - 1):
                if j + 1 < NCH:
                    xb_t[j + 1] = xbp.tile(
                        [128, CT, 512], F16, tag="xb", name=f"xb{j + 1}"
                    )
                    nc.sync.dma_start(out=xb_t[j + 1], in_=xb_d[:, j + 1])
                aj = j - 1
                segs = [(h, s) for h in range(HPC) for s in range(aj + 1)]
                s_i = 0
                av_i = 0
                for gi, (kind, a) in enumerate(chunk_groups(j)):
                    (qkv_group if kind == "qkv" else v_group)(j, a)
                    while av_i < gi * len(segs) // 12:
                        h, s = segs[av_i]
                        attn_av(aj, h, s, final=(s == aj))
                        av_i += 1
                    while s_i < (gi + 1) * len(segs) // 12:
                        attn_s(aj, *segs[s_i])
                        s_i += 1
                while av_i < len(segs):
                    h, s = segs[av_i]
                    attn_av(aj, h, s, final=(s == aj))
                    av_i += 1
                if j >= 2:
                    for tt in range(4 * (j - 2), 4 * (j - 2) + 4):
                        proj_tile(tt)

            # ---- chunk 3: Q groups first, then K/V, x attn(2) segments.
            # attn(3) segs 0..2 only need K/V from chunks 0..2, so after the
            # Q groups the whole of attn(3) interleaves here, leaving only
            # pure projection work for the tail.
            j = NCH - 1
            aj = j - 1
            segs = [(h, s) for h in range(HPC) for s in range(aj + 1)]
            s_i = 0
            av_i = 0
            c3_groups = [("qkv", h) for h in range(HPC)]
            c3_groups += [("qkv", ct) for ct in (4, 5, 6, 7)]
            c3_groups += [("v", tt) for tt in range(4)]
            for gi, (kind, a) in enumerate(c3_groups):
                (qkv_group if kind == "qkv" else v_group)(j, a)
                while av_i < gi * len(segs) // 12:
                    h, s = segs[av_i]
                    attn_av(aj, h, s, final=(s == aj))
                    av_i += 1
                while s_i < (gi + 1) * len(segs) // 12:
                    attn_s(aj, *segs[s_i])
                    s_i += 1
            while av_i < len(segs):
                h, s = segs[av_i]
                attn_av(aj, h, s, final=(s == aj))
                av_i += 1
            # attn(3) heads with a proj(1) quarter-tile after every segment
            # as PE cover for the ACT exp backlog
            pieces = [
                (4 * (NCH - 3) + t2, cc) for t2 in range(4) for cc in range(4)
            ]
            pi = 0
            prev = None
            for h in range(HPC):
                for s in range(NCH):
                    attn_s(NCH - 1, h, s)
                    if pi < len(pieces):
                        proj_piece(*pieces[pi])
                        pi += 1
                    if prev is not None:
                        ph, psg = prev
                        attn_av(NCH - 1, ph, psg, final=(psg == NCH - 1))
                    prev = (h, s)
            attn_av(NCH - 1, prev[0], prev[1], final=True)
            # pure projection tail: proj(2) then proj(3)
            for tt in range(4 * (NCH - 2), 4 * NCH):
                proj_tile(tt, fine_store=(tt == 4 * NCH - 1))

    nc.compile()
    return nc
